# revision 46
# baseline (speedup 1.0000x reference)
"""Trainium2 Bass kernel for a pre-norm transformer block (B=16, N=1024, C=768).

Sharding: data-parallel over batch across 8 NeuronCores (2 items/core).
Precision plan (validated via numpy emulation, target HW rel err ~1.3e-2):
  - qkv/proj fp8 DoubleRow (host-quantized weights at power-of-2 scales)
  - q,k stored fp8 at 2^SA; scores via fp8 DR with half-zero-padded kpad
    (lhsT [64,2,128]: head A in slot0/parts0-63, head B slot1/parts64-127)
  - exp on ACT with constant shift (softmax-invariant) so e fits fp8;
    e8 stored kt-paired [128,2,N] for DR p@v; v fp8 kt-paired with ones col
  - fc1 fp8 DR 3-chain (h8@W8 + hr8@W8 + h8@Wr8), fc2 fp8 DR 3-chain
    (g8@W8 + gr8@W8 + g8@Wr8)
  - LN via bn_stats/bn_aggr (DVE) + Rsqrt (ACT); evacuations on DVE/Pool
"""

import numpy as np
import ml_dtypes
from contextlib import ExitStack

import concourse.bass as bass
import concourse.mybir as mybir
import concourse.tile as tile
from concourse import bacc
from concourse.masks import make_identity
from concourse.bass_utils import run_bass_kernel_spmd

DT = mybir.dt
F32, F32R, BF16, F8 = DT.float32, DT.float32r, DT.bfloat16, DT.float8e4
PM = mybir.MatmulPerfMode
NP8 = ml_dtypes.float8_e4m3
AF = mybir.ActivationFunctionType
ALU = mybir.AluOpType

B, N, C = 16, 1024, 768
HEADS, HD, HID = 12, 64, 3072
EPS = 1e-5
NCORES = 8
BL = B // NCORES            # 2 batch items per core
T = BL * N                  # 2048 tokens per core
TT = T // 128               # 16 token tiles
KC = C // 128               # 6 contraction chunks over C
HC = HID // 128             # 24 hidden chunks
NPB = N // 128              # 8 token tiles per batch item
HP = HEADS // 2             # 6 head pairs
SA = 3                      # q,k fp8 scale exponent (values x 2^SA)
SHIFT = 4.0                 # exp overflow shift (softmax-invariant)

_CACHE = {}


def _build(flags):
    (ln1_aff, ln2_aff, qkv_bias, proj_bias, fc1_bias, fc2_bias,
     s_qkv, s_proj, s_fc1, s_fc2) = flags
    iq, ip, i1, i2 = (2.0 ** -s for s in (s_qkv, s_proj, s_fc1, s_fc2))
    sqk = 2.0 ** (SA - s_qkv)            # q,k psum -> fp8 evac scale
    sc_exp = (HD ** -0.5) * 4.0 ** -SA   # exp scale on fp8 scores psum
    nc = bacc.Bacc("TRN2", target_bir_lowering=False, debug=False,
                   num_devices=NCORES)

    x_d = nc.dram_tensor("x", [BL, N, C], F32, kind="ExternalInput")
    ln1_g_d = nc.dram_tensor("ln1_g", [C], F32, kind="ExternalInput")
    ln1_b_d = nc.dram_tensor("ln1_b", [C], F32, kind="ExternalInput")
    qkv_b_d = nc.dram_tensor("qkv_b", [3 * C], F32, kind="ExternalInput")
    proj_b_d = nc.dram_tensor("proj_b", [C], F32, kind="ExternalInput")
    ln2_g_d = nc.dram_tensor("ln2_g", [C], F32, kind="ExternalInput")
    ln2_b_d = nc.dram_tensor("ln2_b", [C], F32, kind="ExternalInput")
    fc1_b_d = nc.dram_tensor("fc1_b", [HID], F32, kind="ExternalInput")
    fc2_b_d = nc.dram_tensor("fc2_b", [C], F32, kind="ExternalInput")
    qkvw8_d = nc.dram_tensor("qkv_w8", [KC // 2, 128, 2, 2 * C], F8, kind="ExternalInput")
    vw8_d = nc.dram_tensor("v_w8", [KC // 2, 128, 2, C], F8, kind="ExternalInput")
    pw8_d = nc.dram_tensor("proj_w8", [KC // 2, 128, 2, C], F8, kind="ExternalInput")
    w18_d = nc.dram_tensor("fc1_w8", [KC // 2, 128, 2, HID], F8, kind="ExternalInput")
    w1r8_d = nc.dram_tensor("fc1_wr8", [KC // 2, 128, 2, HID], F8, kind="ExternalInput")
    w28_d = nc.dram_tensor("fc2_w8", [HC // 2, 128, 2, C], F8, kind="ExternalInput")
    w2r8_d = nc.dram_tensor("fc2_wr8", [HC // 2, 128, 2, C], F8, kind="ExternalInput")
    out_d = nc.dram_tensor("out", [BL, N, C], F32, kind="ExternalOutput")

    rec_d = nc.dram_tensor("rec_scratch", [BL, HP, 2, 2, 512], F32R)

    x_r = x_d.ap().flatten_outer_dims().rearrange("(t p) c -> t p c", p=128)
    out_r = out_d.ap().flatten_outer_dims().rearrange("(t p) c -> t p c", p=128)

    def bcast_row(dram_ap):
        return bass.AP(tensor=dram_ap.tensor, offset=dram_ap.offset,
                       ap=[[0, 128]] + [list(p) for p in dram_ap.ap])

    with tile.TileContext(nc) as tc, ExitStack() as top:
        const = top.enter_context(tc.tile_pool(name="const", bufs=1))
        identity_f = const.tile([128, 128], F32)
        make_identity(nc, identity_f)
        identity = const.tile([128, 128], F32R)
        nc.vector.tensor_copy(identity, identity_f)
        nshift = const.tile([128, 1], F32)
        nc.vector.memset(nshift, -SHIFT)

        if ln1_aff:
            ln1g_bc = const.tile([128, C], F32)
            nc.sync.dma_start(out=ln1g_bc, in_=bcast_row(ln1_g_d[:]))
            ln1b_bc = const.tile([128, C], F32)
            nc.sync.dma_start(out=ln1b_bc, in_=bcast_row(ln1_b_d[:]))
        if ln2_aff:
            ln2g_bc = const.tile([128, C], F32)
            nc.sync.dma_start(out=ln2g_bc, in_=bcast_row(ln2_g_d[:]))
            ln2b_bc = const.tile([128, C], F32)
            nc.sync.dma_start(out=ln2b_bc, in_=bcast_row(ln2_b_d[:]))
        if qkv_bias:
            # per-feature bias for q,k rows; scaled by 2^SA to match fp8 q,k
            qkvb_raw = const.tile([128, 2 * KC], F32)
            nc.sync.dma_start(out=qkvb_raw,
                              in_=qkv_b_d[0:2 * C].rearrange("(c p) -> p c", p=128))
            qkvb_pp = const.tile([128, 2 * KC], F32)
            nc.vector.tensor_scalar_mul(qkvb_pp, qkvb_raw, 2.0 ** SA)
            vb_bc = const.tile([128, C], F32)
            nc.sync.dma_start(out=vb_bc, in_=bcast_row(qkv_b_d[2 * C:3 * C]))
        if proj_bias:
            projb_bc = const.tile([128, C], F32)
            nc.sync.dma_start(out=projb_bc, in_=bcast_row(proj_b_d[:]))
        if fc1_bias:
            fc1b_pp = const.tile([128, HC], F32)
            nc.sync.dma_start(out=fc1b_pp,
                              in_=fc1_b_d[:].rearrange("(c p) -> p c", p=128))
        if fc2_bias:
            fc2b_bc = const.tile([128, C], F32)
            nc.sync.dma_start(out=fc2b_bc, in_=bcast_row(fc2_b_d[:]))

        # x2 tiles (bf16) stay SBUF-resident from proj through the MLP
        x2s = ExitStack()
        x2pool = x2s.enter_context(tc.tile_pool(name="x2res", bufs=1))
        x2t = [x2pool.tile([128, C], BF16, name=f"x2t{t}") for t in range(TT)]

        # MLP fc1 weights: pool lives whole kernel; DMA overlaps phase A/B
        wmlp = ExitStack()
        w1pool = wmlp.enter_context(tc.tile_pool(name="w1", bufs=1))
        w18, w1r8 = [], []
        for j in range(KC // 2):
            wa = w1pool.tile([128, 2, HID], F8, name=f"w1_{j}")
            nc.gpsimd.dma_start(out=wa, in_=w18_d[j])
            w18.append(wa)
            wb = w1pool.tile([128, 2, HID], F8, name=f"w1r_{j}")
            nc.gpsimd.dma_start(out=wb, in_=w1r8_d[j])
            w1r8.append(wb)
        # attention-wide pools (freed after the MLP tail)
        attn_sc = ExitStack()
        epool = attn_sc.enter_context(tc.tile_pool(name="e8", bufs=9))
        rpool = attn_sc.enter_context(tc.tile_pool(name="rec", bufs=2))
        bcpool = attn_sc.enter_context(tc.tile_pool(name="bcp", bufs=3))
        xres = attn_sc.enter_context(tc.tile_pool(name="xres", bufs=2))
        statsE = attn_sc.enter_context(tc.tile_pool(name="statsE", bufs=1))
        stats2 = attn_sc.enter_context(tc.tile_pool(name="stats2", bufs=4))
        mid = ExitStack()
        pwpool = mid.enter_context(tc.tile_pool(name="pw", bufs=1))
        pw8 = [pwpool.tile([128, 2, C], F8, name=f"pw{s}") for s in range(KC // 2)]

        # per-batch-item attention operands; created b1-first so b0 (on top
        # of the pool stack) can be freed right after attn(b0)
        bstk = [ExitStack() for _ in range(BL)]
        bstk1_late = ExitStack()
        vp_b, qTd_b, kpad_b, oT8_b = [None] * BL, [None] * BL, [None] * BL, [None] * BL
        for b in reversed(range(BL)):
            ostk = bstk1_late if b == 1 else bstk[b]
            opool = ostk.enter_context(tc.tile_pool(name=f"oT{b}", bufs=1))
            oT8_b[b] = [opool.tile([128, 2, N], F8, name=f"oT{b}_{j}")
                        for j in range(KC // 2)]
            vpool = bstk[b].enter_context(tc.tile_pool(name=f"vp{b}", bufs=1))
            vp = [vpool.tile([128, 2, HEADS, HD + 4], F8, name=f"vp{b}_{t}")
                  for t in range(NPB // 2)]
            vp_b[b] = vp
            qpool = bstk[b].enter_context(tc.tile_pool(name=f"qTd{b}", bufs=1))
            qTd = [qpool.tile([128, 2, N], F8, name=f"qTd{b}_{i}")
                   for i in range(HP)]
            qTd_b[b] = qTd
            kpool = bstk[b].enter_context(tc.tile_pool(name=f"kpad{b}", bufs=1))
            kpad = [kpool.tile([128, NPB, 2, 128], F8, name=f"kp{b}_{i}")
                    for i in range(HP)]
            kpad_b[b] = kpad
            # zero dont-care slot-1 once (q,k live dense in slot 0; garbage
            # in slot 1 could be NaN-pattern and 0*NaN = NaN in the matmul)
            for t in range(NPB // 2):
                nc.gpsimd.memset(vp[t][:, :, :, HD:HD + 1], 1.0)
                nc.gpsimd.memset(vp[t][:, :, :, HD + 1:HD + 4], 0.0)
            for i in range(HP):
                eng = nc.vector if i % 2 == 0 else nc.gpsimd
                eng.memset(qTd[i][:, 1, :], 0.0)
                eng.memset(kpad[i][:, :, 1, :], 0.0)

        hT_stack = ExitStack()
        hT_pool = hT_stack.enter_context(tc.tile_pool(name="hT", bufs=1))
        hT8 = hT_pool.tile([128, KC, T], F8, name="hT8")

        def ln_stats(stats, x_t):
            """bn_stats-based mean/var -> (negmu [128,1], rstd [128,1])."""
            st = stats.tile([128, 12], F32, tag="st")
            nc.vector.bn_stats(st[:, 0:6], x_t[:, 0:C // 2])
            nc.vector.bn_stats(st[:, 6:12], x_t[:, C // 2:C])
            mv = stats.tile([128, 2], F32, tag="mv")
            nc.vector.bn_aggr(mv, st)
            vare = stats.tile([128, 1], F32, tag="vare")
            nc.vector.tensor_scalar_add(vare, mv[:, 1:2], EPS)
            std = stats.tile([128, 1], F32, tag="std")
            nc.scalar.sqrt(std, vare)
            rstd = stats.tile([128, 1], F32, tag="rstd")
            nc.vector.reciprocal(rstd, std)
            negmu = stats.tile([128, 1], F32, tag="negmu")
            nc.vector.tensor_scalar_mul(negmu, mv[:, 0:1], -1.0)
            return negmu, rstd

        # ---------------- Phase A: LN1 + transpose -> hT8 ----------------
        with nc.named_scope("ln1"), ExitStack() as ph:
            xpool = ph.enter_context(tc.tile_pool(name="xa", bufs=3))
            stats = ph.enter_context(tc.tile_pool(name="stats", bufs=8))
            hpool = ph.enter_context(tc.tile_pool(name="h", bufs=2))
            pst = ph.enter_context(tc.tile_pool(name="pst", bufs=4, space="PSUM"))
            for t in range(TT):
                x_t = xpool.tile([128, C], F32, tag="x")
                nc.sync.dma_start(out=x_t, in_=x_r[t])
                negmu, rstd = ln_stats(stats, x_t)
                h_t = hpool.tile([128, C], F32R, tag="h")
                nc.vector.tensor_scalar(h_t, x_t, negmu, rstd,
                                        op0=ALU.add, op1=ALU.mult)
                if ln1_aff:
                    nc.vector.tensor_tensor(h_t, h_t, ln1g_bc, op=ALU.mult)
                    nc.vector.tensor_tensor(h_t, h_t, ln1b_bc, op=ALU.add)
                for g3 in range(2):
                    ps3 = pst.tile([128, 384], F32R, tag="pt")
                    for c3 in range(3):
                        f = g3 * 3 + c3
                        nc.tensor.transpose(ps3[:, c3 * 128:(c3 + 1) * 128],
                                            h_t[:, f * 128:(f + 1) * 128], identity)
                    dst = hT8[:, g3 * 3:(g3 + 1) * 3, t * 128:(t + 1) * 128]
                    if t % 2 == 0:
                        nc.vector.tensor_copy(dst, ps3)
                    else:
                        nc.gpsimd.tensor_copy(dst, ps3)

        wpool = hT_stack.enter_context(tc.tile_pool(name="qkvw", bufs=1))
        wqk8 = [wpool.tile([128, 2, 2 * C], F8, name=f"wqk{j}") for j in range(KC // 2)]
        wv8 = [wpool.tile([128, 2, C], F8, name=f"wv{j}") for j in range(KC // 2)]
        for j in range(KC // 2):
            nc.sync.dma_start(out=wqk8[j], in_=qkvw8_d[j])
            nc.sync.dma_start(out=wv8[j], in_=vw8_d[j])

        # ---------------- Phase B: qkv ----------------
        with nc.named_scope("qkv"), ExitStack() as ph:
            psqk = ph.enter_context(tc.tile_pool(name="psqk", bufs=6, space="PSUM"))
            # q,k feature-major, fp8 at 2^SA; q dense-stacked, k half-zero padded
            for fc in range(2 * KC):
                for tc4 in range(T // 512):
                    ps = psqk.tile([128, 512], F32, tag="ps")
                    for j in range(KC // 2):
                        nc.tensor.matmul(ps, wqk8[j][:, :, fc * 128:(fc + 1) * 128],
                                         hT8[:, 2 * j:2 * j + 2, tc4 * 512:(tc4 + 1) * 512],
                                         start=(j == 0), stop=(j == KC // 2 - 1),
                                         perf_mode=PM.DoubleRow)
                    b = tc4 // 2
                    if fc < KC:      # q chunk: heads (2fc, 2fc+1) dense slot 0
                        hp = fc
                        ls = slice((tc4 % 2) * 512, (tc4 % 2) * 512 + 512)
                        dst = qTd_b[b][hp][:, 0, ls]
                    else:            # k chunk: heads (2hp, 2hp+1), keys on free
                        hp = fc - KC
                        kt0 = (tc4 % 2) * 4
                        dst = kpad_b[b][hp][:, kt0:kt0 + 4, 0, :]
                    eng = nc.vector if (fc + tc4) % 2 == 0 else nc.gpsimd
                    if qkv_bias:
                        eng.tensor_scalar(dst, ps, sqk, qkvb_pp[:, fc:fc + 1],
                                          op0=ALU.mult, op1=ALU.add)
                    else:
                        eng.tensor_scalar_mul(dst, ps, sqk)
            # v token-major -> vp fp8 (kt-paired), natural scale
            for t in range(TT):
                for off, h_lo, h_hi in ((0, 0, 6), (384, 6, 12)):
                    ps = psqk.tile([128, 512], F32, tag="ps")
                    for j in range(KC // 2):
                        nc.tensor.matmul(ps[:, 0:384],
                                         hT8[:, 2 * j:2 * j + 2, t * 128:(t + 1) * 128],
                                         wv8[j][:, :, off:off + 384],
                                         start=(j == 0), stop=(j == KC // 2 - 1),
                                         perf_mode=PM.DoubleRow)
                    dst = vp_b[t // NPB][(t % NPB) // 2][:, t % 2, h_lo:h_hi, 0:HD]
                    eng = nc.vector if t % 2 == 0 else nc.gpsimd
                    if qkv_bias:
                        eng.scalar_tensor_tensor(dst, ps[:, 0:384], iq,
                                                 vb_bc[:, off:off + 384],
                                                 op0=ALU.mult, op1=ALU.add)
                    else:
                        eng.tensor_scalar_mul(dst, ps[:, 0:384], iq)

        for j in range(KC // 2):
            nc.sync.dma_start(out=pw8[j], in_=pw8_d[j])

        # ------- Phases C+E: attention pipelined with the MLP -------
        hT_stack.close()
        pssc_sc = ExitStack()
        pssc = pssc_sc.enter_context(tc.tile_pool(name="pssc", bufs=2, space="PSUM"))
        ln2s = [None] * TT

        def attn_hp(b, hp, psov, pre_pieces, mid_pieces):
            kp = kpad_b[b][hp]
            qd = qTd_b[b][hp]
            oT8 = oT8_b[b]
            for p in pre_pieces:
                p()
            eAs, eBs = [], []
            for kt in range(NPB):
                psS = pssc.tile([128, N], F32, tag="s")
                psB = pssc.tile([128, N], F32, tag="s")
                for qc in range(2):
                    qs = slice(qc * 512, (qc + 1) * 512)
                    nc.tensor.matmul(psS[:, qs], kp[0:64, kt, 0, :],
                                     qd[0:64, 0, qs], start=True, stop=True,
                                     tile_position=(0, 0))
                    nc.tensor.matmul(psB[:, qs], kp[64:128, kt, 0, :],
                                     qd[64:128, 0, qs], start=True, stop=True,
                                     tile_position=(64, 0))
                if kt % 2 == 0:
                    eA_t = epool.tile([128, 2, N], F8, tag="e8", name="eA")
                    eB_t = epool.tile([128, 2, N], F8, tag="e8", name="eB")
                    eAs.append(eA_t)
                    eBs.append(eB_t)
                nc.scalar.activation(eAs[kt // 2][:, kt % 2, :], psS,
                                     AF.Exp, scale=sc_exp, bias=nshift)
                nc.scalar.activation(eBs[kt // 2][:, kt % 2, :], psB,
                                     AF.Exp, scale=sc_exp, bias=nshift)
            for qc in range(2):
                qs = slice(qc * 512, (qc + 1) * 512)
                psoA = psov.tile([128, 512], F32, tag="o")
                psoB = psov.tile([128, 512], F32, tag="o")
                for ps_o, et, h in ((psoA, eAs, 2 * hp), (psoB, eBs, 2 * hp + 1)):
                    for ktp in range(NPB // 2):
                        nc.tensor.matmul(ps_o[0:HD + 4, :],
                                         vp_b[b][ktp][:, :, h, :],
                                         et[ktp][:, :, qs],
                                         start=(ktp == 0), stop=(ktp == 3),
                                         perf_mode=PM.DoubleRow)
                recA = rpool.tile([1, 512], F32R, tag="rA")
                recB = rpool.tile([1, 512], F32R, tag="rB")
                with nc.allow_low_precision(reason="fp32r is fp32 bits"):
                    nc.vector.reciprocal(recA, psoA[HD:HD + 1, :])
                    nc.vector.reciprocal(recB, psoB[HD:HD + 1, :])
                nc.sync.dma_start(out=rec_d[b, hp, qc, 0], in_=recA)
                nc.sync.dma_start(out=rec_d[b, hp, qc, 1], in_=recB)
                bc_sb = bcpool.tile([128, 512], F32R, tag="bc")
                rAd = rec_d[b, hp, qc, 0]
                rBd = rec_d[b, hp, qc, 1]
                nc.sync.dma_start(
                    out=bc_sb[0:64, :],
                    in_=bass.AP(tensor=rAd.tensor, offset=rAd.offset,
                                ap=[[0, 64]] + [list(p) for p in rAd.ap]))
                nc.sync.dma_start(
                    out=bc_sb[64:128, :],
                    in_=bass.AP(tensor=rBd.tensor, offset=rBd.offset,
                                ap=[[0, 64]] + [list(p) for p in rBd.ap]))
                nc.vector.tensor_tensor(
                    oT8[hp // 2][0:64, hp % 2, qs], psoA[0:HD, :],
                    bc_sb[0:64, :], op=ALU.mult)
                nc.gpsimd.tensor_tensor(
                    oT8[hp // 2][64:128, hp % 2, qs], psoB[0:HD, :],
                    bc_sb[64:128, :], op=ALU.mult)
                if qc == 0:
                    for p in mid_pieces:
                        p()

        def proj_b(b, psov, ptag="o"):
            for lt in range(NPB):
                t = b * NPB + lt
                x_t = xres.tile([128, C], F32, tag="xr")
                nc.sync.dma_start(out=x_t, in_=x_r[t])
                for off, sz in ((0, 384), (384, 384)):
                    ps = psov.tile([128, 512], F32, tag=ptag)
                    for j in range(KC // 2):
                        nc.tensor.matmul(ps[:, 0:sz],
                                         oT8_b[b][j][:, :, lt * 128:(lt + 1) * 128],
                                         pw8[j][:, :, off:off + sz],
                                         start=(j == 0), stop=(j == KC // 2 - 1),
                                         perf_mode=PM.DoubleRow)
                    nc.vector.scalar_tensor_tensor(
                        x2t[t][:, off:off + sz], ps[:, 0:sz], ip,
                        x_t[:, off:off + sz], op0=ALU.mult, op1=ALU.add)
                if proj_bias:
                    nc.vector.tensor_tensor(x2t[t], x2t[t], projb_bc, op=ALU.add)

        def stats_b(b):
            for t in range(b * NPB, (b + 1) * NPB):
                st = statsE.tile([128, 12], F32, name=f"stE{t}")
                nc.vector.bn_stats(st[:, 0:6], x2t[t][:, 0:C // 2])
                nc.vector.bn_stats(st[:, 6:12], x2t[t][:, C // 2:C])
                mv = statsE.tile([128, 2], F32, name=f"mvE{t}")
                nc.vector.bn_aggr(mv, st)
                vare = stats2.tile([128, 1], F32, tag="vare")
                nc.vector.tensor_scalar_add(vare, mv[:, 1:2], EPS)
                std = stats2.tile([128, 1], F32, tag="sdE")
                nc.scalar.sqrt(std, vare)
                rstd = statsE.tile([128, 1], F32, name=f"rsE{t}")
                nc.vector.reciprocal(rstd, std)
                negmu = statsE.tile([128, 1], F32, name=f"nmE{t}")
                nc.vector.tensor_scalar_mul(negmu, mv[:, 0:1], -1.0)
                ln2s[t] = (negmu, rstd)

        # ---- attention, both batch items (serial, 4-bank psov) ----
        ph0 = ExitStack()
        psov0 = ph0.enter_context(tc.tile_pool(name="psov0", bufs=4,
                                               space="PSUM"))
        with nc.named_scope("attn0"):
            for hp in range(HP):
                attn_hp(0, hp, psov0, [], [])
            proj_b(0, psov0)
        bstk[0].close()
        stats_b(0)
        with nc.named_scope("attn1"):
            for hp in range(HP):
                attn_hp(1, hp, psov0, [], [])
        bstk[1].close()
        ph0.close()
        pssc_sc.close()

        # ---- MLP over all 4 chunks ----
        CH = 4
        CT = T // CH // 128
        phW = ExitStack()
        w2pool = phW.enter_context(tc.tile_pool(name="w2", bufs=1))
        w2_8, w2r8 = [], []
        for u in range(HC // 2):
            wa = w2pool.tile([128, 2, C], F8, name=f"w2_{u}")
            nc.gpsimd.dma_start(out=wa, in_=w28_d[u])
            w2_8.append(wa)
            wb = w2pool.tile([128, 2, C], F8, name=f"w2r_{u}")
            nc.gpsimd.dma_start(out=wb, in_=w2r8_d[u])
            w2r8.append(wb)
        psf1 = phW.enter_context(tc.tile_pool(name="psf1", bufs=3, space="PSUM"))
        psfT = phW.enter_context(tc.tile_pool(name="psfT", bufs=2, space="PSUM"))
        psf2 = phW.enter_context(tc.tile_pool(name="psf2", bufs=2, space="PSUM"))
        h2pool = phW.enter_context(tc.tile_pool(name="h2", bufs=3))
        h2Tpool = phW.enter_context(tc.tile_pool(name="h2T", bufs=2))
        hrpool = phW.enter_context(tc.tile_pool(name="hr8", bufs=2))
        gpool = phW.enter_context(tc.tile_pool(name="gT", bufs=6))
        g8pool = phW.enter_context(tc.tile_pool(name="g8", bufs=13))
        gr8pool = phW.enter_context(tc.tile_pool(name="gr8", bufs=13))
        outpool = phW.enter_context(tc.tile_pool(name="outp", bufs=2))
        h2T_c, hr_c, g8_c, gr8_c = {}, {}, {}, {}
        def mlp_trans(ch, pspool):
            h2T_c[ch] = h2Tpool.tile([128, KC, CT * 128], F8, tag="h2T",
                                     name=f"h2T{ch}")
            hr_c[ch] = hrpool.tile([128, KC, CT * 128], F8, tag="hr8",
                                   name=f"hr{ch}")
            for lt in range(CT):
                t = ch * CT + lt
                negmu, rstd = ln2s[t]
                h2 = h2pool.tile([128, C], F32R, tag="h2")
                nc.vector.tensor_scalar(h2, x2t[t], negmu, rstd,
                                        op0=ALU.add, op1=ALU.mult)
                if ln2_aff:
                    nc.vector.tensor_tensor(h2, h2, ln2g_bc, op=ALU.mult)
                    nc.vector.tensor_tensor(h2, h2, ln2b_bc, op=ALU.add)
                for g3 in range(2):
                    ps3 = pspool.tile([128, 384], F32R, tag="o")
                    for c3 in range(3):
                        f = g3 * 3 + c3
                        nc.tensor.transpose(ps3[:, c3 * 128:(c3 + 1) * 128],
                                            h2[:, f * 128:(f + 1) * 128], identity)
                    dst8 = h2T_c[ch][:, g3 * 3:(g3 + 1) * 3,
                                     lt * 128:(lt + 1) * 128]
                    if lt % 2 == 0:
                        nc.vector.tensor_copy(dst8, ps3)
                    else:
                        nc.gpsimd.tensor_copy(dst8, ps3)
                    dstr = hr_c[ch][:, g3 * 3:(g3 + 1) * 3,
                                    lt * 128:(lt + 1) * 128]
                    nc.vector.scalar_tensor_tensor(dstr, ps3, 1.0, dst8,
                                                   op0=ALU.mult, op1=ALU.subtract)

        def mlp_fc1(ch, lo, hi):
            if lo == 0:
                g8_c[ch] = [g8pool.tile([128, 2, 512], F8, tag="g8",
                                        name=f"g8_{ch}_{u}")
                            for u in range(HC // 2)]
                gr8_c[ch] = [gr8pool.tile([128, 2, 512], F8, tag="gr8",
                                          name=f"gr8_{ch}_{u}")
                             for u in range(HC // 2)]
            for sidx in range(lo, hi):
                half, hc = sidx // 12, sidx % 12
                ps = psf1.tile([128, 512], F32, tag="f1")
                off = half * (HID // 2) + hc * 128
                nmm = 3 * (KC // 2)
                k = 0
                for j in range(KC // 2):
                    for lh, rh in ((w18[j], h2T_c[ch]), (w18[j], hr_c[ch]),
                                   (w1r8[j], h2T_c[ch])):
                        nc.tensor.matmul(ps, lh[:, :, off:off + 128],
                                         rh[:, 2 * j:2 * j + 2, :],
                                         start=(k == 0), stop=(k == nmm - 1),
                                         perf_mode=PM.DoubleRow)
                        k += 1
                g_t = gpool.tile([128, 512], BF16, tag="g")
                if fc1_bias:
                    nc.scalar.activation(g_t, ps, AF.Gelu, scale=i1,
                                         bias=fc1b_pp[:, sidx:sidx + 1])
                else:
                    nc.scalar.activation(g_t, ps, AF.Gelu, scale=i1)
                g8s = g8_c[ch][sidx // 2][:, sidx % 2, :]
                nc.gpsimd.tensor_copy(g8s, g_t)
                nc.vector.tensor_tensor(gr8_c[ch][sidx // 2][:, sidx % 2, :],
                                        g_t, g8s, op=ALU.subtract)

        def mlp_fc2(ch, pspool):
            g8p, gr8p = g8_c[ch], gr8_c[ch]
            for lt in range(CT):
                t = ch * CT + lt
                out_sb = outpool.tile([128, C], F32, tag="out")
                for off, sz in ((0, 384), (384, 384)):
                    ps = pspool.tile([128, 384], F32, tag="f2")
                    nmm = 3 * (HC // 2)
                    k = 0
                    for u in range(HC // 2):
                        ts = slice(lt * 128, (lt + 1) * 128)
                        for lh, rh in ((g8p[u], w2_8[u]), (gr8p[u], w2_8[u]),
                                       (g8p[u], w2r8[u])):
                            nc.tensor.matmul(ps[:, 0:sz], lh[:, :, ts],
                                             rh[:, :, off:off + sz],
                                             start=(k == 0), stop=(k == nmm - 1),
                                             perf_mode=PM.DoubleRow)
                            k += 1
                    nc.vector.scalar_tensor_tensor(
                        out_sb[:, off:off + sz], ps[:, 0:sz], i2,
                        x2t[t][:, off:off + sz], op0=ALU.mult, op1=ALU.add)
                if fc2_bias:
                    nc.vector.tensor_tensor(out_sb, out_sb, fc2b_bc, op=ALU.add)
                nc.sync.dma_start(out=out_r[t], in_=out_sb)

        with nc.named_scope("mlp"):
            mlp_trans(0, psfT)
            proj_b(1, psf1, ptag="f1")
            stats_b(1)
            mlp_fc1(0, 0, 24)
            mlp_fc2(0, psf2)
            for ch in (1, 2, 3):
                mlp_trans(ch, psfT)
                mlp_fc1(ch, 0, 24)
                mlp_fc2(ch, psf2)
        phW.close()
        bstk1_late.close()
        mid.close()
        attn_sc.close()
        wmlp.close()
        x2s.close()

    nc.finalize()
    return nc


def _get_nc(flags):
    if flags not in _CACHE:
        _CACHE[flags] = _build(flags)
    return _CACHE[flags]


def _scale_for(w):
    return int(np.clip(np.floor(np.log2(200.0 / max(abs(float(w.max())),
                                                    abs(float(w.min())), 1e-9))),
                       0, 14))


def _pack_rows(w, s, residual=False):
    # [K, M] f32 -> [K//256, 128, 2, M] fp8 at scale 2**s (+ residual fp8)
    K, M = w.shape
    ws = (w * float(2 ** s)).reshape(K // 256, 2, 128, M).transpose(0, 2, 1, 3)
    w8 = np.ascontiguousarray(ws).astype(NP8)
    if not residual:
        return w8
    wr8 = np.ascontiguousarray(ws - w8.astype(np.float32)).astype(NP8)
    return w8, wr8


def kernel(**inputs):
    inp = {k: np.ascontiguousarray(np.asarray(v, dtype=np.float32))
           for k, v in inputs.items()}
    s_qkv = _scale_for(inp["qkv_w"])
    s_proj = _scale_for(inp["proj_w"])
    s_fc1 = _scale_for(inp["fc1_w"])
    s_fc2 = _scale_for(inp["fc2_w"])
    flags = (
        not (np.all(inp["ln1_g"] == 1.0) and np.all(inp["ln1_b"] == 0.0)),
        not (np.all(inp["ln2_g"] == 1.0) and np.all(inp["ln2_b"] == 0.0)),
        bool(np.any(inp["qkv_b"] != 0.0)),
        bool(np.any(inp["proj_b"] != 0.0)),
        bool(np.any(inp["fc1_b"] != 0.0)),
        bool(np.any(inp["fc2_b"] != 0.0)),
        s_qkv, s_proj, s_fc1, s_fc2,
    )
    nc = _get_nc(flags)
    x = inp["x"]
    shared = {k: v for k, v in inp.items()
              if k not in ("x", "qkv_w", "proj_w", "fc1_w", "fc2_w")}
    shared["qkv_w8"] = _pack_rows(inp["qkv_w"][:, 0:2 * C], s_qkv)
    shared["v_w8"] = _pack_rows(inp["qkv_w"][:, 2 * C:3 * C], s_qkv)
    shared["proj_w8"] = _pack_rows(inp["proj_w"], s_proj)
    shared["fc1_w8"], shared["fc1_wr8"] = _pack_rows(inp["fc1_w"], s_fc1,
                                                     residual=True)
    shared["fc2_w8"], shared["fc2_wr8"] = _pack_rows(inp["fc2_w"], s_fc2,
                                                     residual=True)
    in_maps = [dict(shared, x=x[i * BL:(i + 1) * BL]) for i in range(NCORES)]
    res = run_bass_kernel_spmd(nc, in_maps, core_ids=list(range(NCORES)))
    out = np.concatenate([res.results[i]["out"] for i in range(NCORES)], axis=0)
    return out.astype(np.float32)


# revision 47
# speedup vs baseline: 1.0014x; 1.0014x over previous
"""Trainium2 Bass kernel for a pre-norm transformer block (B=16, N=1024, C=768).

Sharding: data-parallel over batch across 8 NeuronCores (2 items/core).
Precision plan (validated via numpy emulation, target HW rel err ~1.3e-2):
  - qkv/proj fp8 DoubleRow (host-quantized weights at power-of-2 scales)
  - q,k stored fp8 at 2^SA; scores via fp8 DR with half-zero-padded kpad
    (lhsT [64,2,128]: head A in slot0/parts0-63, head B slot1/parts64-127)
  - exp on ACT with constant shift (softmax-invariant) so e fits fp8;
    e8 stored kt-paired [128,2,N] for DR p@v; v fp8 kt-paired with ones col
  - fc1 fp8 DR 3-chain (h8@W8 + hr8@W8 + h8@Wr8), fc2 fp8 DR 3-chain
    (g8@W8 + gr8@W8 + g8@Wr8)
  - LN via bn_stats/bn_aggr (DVE) + Rsqrt (ACT); evacuations on DVE/Pool
"""

import numpy as np
import ml_dtypes
from contextlib import ExitStack

import concourse.bass as bass
import concourse.mybir as mybir
import concourse.tile as tile
from concourse import bacc
from concourse.masks import make_identity
from concourse.bass_utils import run_bass_kernel_spmd

DT = mybir.dt
F32, F32R, BF16, F8 = DT.float32, DT.float32r, DT.bfloat16, DT.float8e4
PM = mybir.MatmulPerfMode
NP8 = ml_dtypes.float8_e4m3
AF = mybir.ActivationFunctionType
ALU = mybir.AluOpType

B, N, C = 16, 1024, 768
HEADS, HD, HID = 12, 64, 3072
EPS = 1e-5
NCORES = 8
BL = B // NCORES            # 2 batch items per core
T = BL * N                  # 2048 tokens per core
TT = T // 128               # 16 token tiles
KC = C // 128               # 6 contraction chunks over C
HC = HID // 128             # 24 hidden chunks
NPB = N // 128              # 8 token tiles per batch item
HP = HEADS // 2             # 6 head pairs
SA = 3                      # q,k fp8 scale exponent (values x 2^SA)
SHIFT = 4.0                 # exp overflow shift (softmax-invariant)

_CACHE = {}


def _build(flags):
    (ln1_aff, ln2_aff, qkv_bias, proj_bias, fc1_bias, fc2_bias,
     s_qkv, s_proj, s_fc1, s_fc2) = flags
    iq, ip, i1, i2 = (2.0 ** -s for s in (s_qkv, s_proj, s_fc1, s_fc2))
    sqk = 2.0 ** (SA - s_qkv)            # q,k psum -> fp8 evac scale
    sc_exp = (HD ** -0.5) * 4.0 ** -SA   # exp scale on fp8 scores psum
    nc = bacc.Bacc("TRN2", target_bir_lowering=False, debug=False,
                   num_devices=NCORES)

    x_d = nc.dram_tensor("x", [BL, N, C], F32, kind="ExternalInput")
    ln1_g_d = nc.dram_tensor("ln1_g", [C], F32, kind="ExternalInput")
    ln1_b_d = nc.dram_tensor("ln1_b", [C], F32, kind="ExternalInput")
    qkv_b_d = nc.dram_tensor("qkv_b", [3 * C], F32, kind="ExternalInput")
    proj_b_d = nc.dram_tensor("proj_b", [C], F32, kind="ExternalInput")
    ln2_g_d = nc.dram_tensor("ln2_g", [C], F32, kind="ExternalInput")
    ln2_b_d = nc.dram_tensor("ln2_b", [C], F32, kind="ExternalInput")
    fc1_b_d = nc.dram_tensor("fc1_b", [HID], F32, kind="ExternalInput")
    fc2_b_d = nc.dram_tensor("fc2_b", [C], F32, kind="ExternalInput")
    qkvw8_d = nc.dram_tensor("qkv_w8", [KC // 2, 128, 2, 2 * C], F8, kind="ExternalInput")
    vw8_d = nc.dram_tensor("v_w8", [KC // 2, 128, 2, C], F8, kind="ExternalInput")
    pw8_d = nc.dram_tensor("proj_w8", [KC // 2, 128, 2, C], F8, kind="ExternalInput")
    w18_d = nc.dram_tensor("fc1_w8", [KC // 2, 128, 2, HID], F8, kind="ExternalInput")
    w1r8_d = nc.dram_tensor("fc1_wr8", [KC // 2, 128, 2, HID], F8, kind="ExternalInput")
    w28_d = nc.dram_tensor("fc2_w8", [HC // 2, 128, 2, C], F8, kind="ExternalInput")
    w2r8_d = nc.dram_tensor("fc2_wr8", [HC // 2, 128, 2, C], F8, kind="ExternalInput")
    out_d = nc.dram_tensor("out", [BL, N, C], F32, kind="ExternalOutput")

    rec_d = nc.dram_tensor("rec_scratch", [BL, HP, 2, 2, 512], F32R)

    x_r = x_d.ap().flatten_outer_dims().rearrange("(t p) c -> t p c", p=128)
    out_r = out_d.ap().flatten_outer_dims().rearrange("(t p) c -> t p c", p=128)

    def bcast_row(dram_ap):
        return bass.AP(tensor=dram_ap.tensor, offset=dram_ap.offset,
                       ap=[[0, 128]] + [list(p) for p in dram_ap.ap])

    with tile.TileContext(nc) as tc, ExitStack() as top:
        const = top.enter_context(tc.tile_pool(name="const", bufs=1))
        identity_f = const.tile([128, 128], F32)
        make_identity(nc, identity_f)
        identity = const.tile([128, 128], F32R)
        nc.vector.tensor_copy(identity, identity_f)
        nshift = const.tile([128, 1], F32)
        nc.vector.memset(nshift, -SHIFT)

        if ln1_aff:
            ln1g_bc = const.tile([128, C], F32)
            nc.sync.dma_start(out=ln1g_bc, in_=bcast_row(ln1_g_d[:]))
            ln1b_bc = const.tile([128, C], F32)
            nc.sync.dma_start(out=ln1b_bc, in_=bcast_row(ln1_b_d[:]))
        if ln2_aff:
            ln2g_bc = const.tile([128, C], F32)
            nc.sync.dma_start(out=ln2g_bc, in_=bcast_row(ln2_g_d[:]))
            ln2b_bc = const.tile([128, C], F32)
            nc.sync.dma_start(out=ln2b_bc, in_=bcast_row(ln2_b_d[:]))
        if qkv_bias:
            # per-feature bias for q,k rows; scaled by 2^SA to match fp8 q,k
            qkvb_raw = const.tile([128, 2 * KC], F32)
            nc.sync.dma_start(out=qkvb_raw,
                              in_=qkv_b_d[0:2 * C].rearrange("(c p) -> p c", p=128))
            qkvb_pp = const.tile([128, 2 * KC], F32)
            nc.vector.tensor_scalar_mul(qkvb_pp, qkvb_raw, 2.0 ** SA)
            vb_bc = const.tile([128, C], F32)
            nc.sync.dma_start(out=vb_bc, in_=bcast_row(qkv_b_d[2 * C:3 * C]))
        if proj_bias:
            projb_bc = const.tile([128, C], F32)
            nc.sync.dma_start(out=projb_bc, in_=bcast_row(proj_b_d[:]))
        if fc1_bias:
            fc1b_pp = const.tile([128, HC], F32)
            nc.sync.dma_start(out=fc1b_pp,
                              in_=fc1_b_d[:].rearrange("(c p) -> p c", p=128))
        if fc2_bias:
            fc2b_bc = const.tile([128, C], F32)
            nc.sync.dma_start(out=fc2b_bc, in_=bcast_row(fc2_b_d[:]))

        # x2 tiles (bf16) stay SBUF-resident from proj through the MLP
        x2s = ExitStack()
        x2pool = x2s.enter_context(tc.tile_pool(name="x2res", bufs=1))
        x2t = [x2pool.tile([128, C], BF16, name=f"x2t{t}") for t in range(TT)]

        # MLP fc1 weights: pool lives whole kernel; DMA overlaps phase A/B
        wmlp = ExitStack()
        w1pool = wmlp.enter_context(tc.tile_pool(name="w1", bufs=1))
        w18, w1r8 = [], []
        for j in range(KC // 2):
            wa = w1pool.tile([128, 2, HID], F8, name=f"w1_{j}")
            nc.gpsimd.dma_start(out=wa, in_=w18_d[j])
            w18.append(wa)
            wb = w1pool.tile([128, 2, HID], F8, name=f"w1r_{j}")
            nc.gpsimd.dma_start(out=wb, in_=w1r8_d[j])
            w1r8.append(wb)
        # attention-wide pools (freed after the MLP tail)
        attn_sc = ExitStack()
        epool = attn_sc.enter_context(tc.tile_pool(name="e8", bufs=9))
        rpool = attn_sc.enter_context(tc.tile_pool(name="rec", bufs=2))
        bcpool = attn_sc.enter_context(tc.tile_pool(name="bcp", bufs=3))
        xres = attn_sc.enter_context(tc.tile_pool(name="xres", bufs=2))
        statsE = attn_sc.enter_context(tc.tile_pool(name="statsE", bufs=1))
        stats2 = attn_sc.enter_context(tc.tile_pool(name="stats2", bufs=4))
        mid = ExitStack()
        pwpool = mid.enter_context(tc.tile_pool(name="pw", bufs=1))
        pw8 = [pwpool.tile([128, 2, C], F8, name=f"pw{s}") for s in range(KC // 2)]

        # per-batch-item attention operands; created b1-first so b0 (on top
        # of the pool stack) can be freed right after attn(b0)
        bstk = [ExitStack() for _ in range(BL)]
        bstk1_late = ExitStack()
        vp_b, qTd_b, kpad_b, oT8_b = [None] * BL, [None] * BL, [None] * BL, [None] * BL
        for b in reversed(range(BL)):
            ostk = bstk1_late if b == 1 else bstk[b]
            opool = ostk.enter_context(tc.tile_pool(name=f"oT{b}", bufs=1))
            oT8_b[b] = [opool.tile([128, 2, N], F8, name=f"oT{b}_{j}")
                        for j in range(KC // 2)]
            vpool = bstk[b].enter_context(tc.tile_pool(name=f"vp{b}", bufs=1))
            vp = [vpool.tile([128, 2, HEADS, HD + 4], F8, name=f"vp{b}_{t}")
                  for t in range(NPB // 2)]
            vp_b[b] = vp
            qpool = bstk[b].enter_context(tc.tile_pool(name=f"qTd{b}", bufs=1))
            qTd = [qpool.tile([128, 2, N], F8, name=f"qTd{b}_{i}")
                   for i in range(HP)]
            qTd_b[b] = qTd
            kpool = bstk[b].enter_context(tc.tile_pool(name=f"kpad{b}", bufs=1))
            kpad = [kpool.tile([128, NPB, 2, 128], F8, name=f"kp{b}_{i}")
                    for i in range(HP)]
            kpad_b[b] = kpad
            # zero dont-care slot-1 once (q,k live dense in slot 0; garbage
            # in slot 1 could be NaN-pattern and 0*NaN = NaN in the matmul)
            for t in range(NPB // 2):
                nc.gpsimd.memset(vp[t][:, :, :, HD:HD + 1], 1.0)
                nc.gpsimd.memset(vp[t][:, :, :, HD + 1:HD + 4], 0.0)
            for i in range(HP):
                eng = nc.vector if i % 2 == 0 else nc.gpsimd
                eng.memset(qTd[i][:, 1, :], 0.0)
                eng.memset(kpad[i][:, :, 1, :], 0.0)

        hT_stack = ExitStack()
        hT_pool = hT_stack.enter_context(tc.tile_pool(name="hT", bufs=1))
        hT8 = hT_pool.tile([128, KC, T], F8, name="hT8")

        def ln_stats(stats, x_t):
            """bn_stats-based mean/var -> (negmu [128,1], rstd [128,1])."""
            st = stats.tile([128, 12], F32, tag="st")
            nc.vector.bn_stats(st[:, 0:6], x_t[:, 0:C // 2])
            nc.vector.bn_stats(st[:, 6:12], x_t[:, C // 2:C])
            mv = stats.tile([128, 2], F32, tag="mv")
            nc.vector.bn_aggr(mv, st)
            vare = stats.tile([128, 1], F32, tag="vare")
            nc.vector.tensor_scalar_add(vare, mv[:, 1:2], EPS)
            std = stats.tile([128, 1], F32, tag="std")
            nc.scalar.sqrt(std, vare)
            rstd = stats.tile([128, 1], F32, tag="rstd")
            nc.vector.reciprocal(rstd, std)
            negmu = stats.tile([128, 1], F32, tag="negmu")
            nc.vector.tensor_scalar_mul(negmu, mv[:, 0:1], -1.0)
            return negmu, rstd

        # ---------------- Phase A: LN1 + transpose -> hT8 ----------------
        with nc.named_scope("ln1"), ExitStack() as ph:
            xpool = ph.enter_context(tc.tile_pool(name="xa", bufs=3))
            stats = ph.enter_context(tc.tile_pool(name="stats", bufs=8))
            hpool = ph.enter_context(tc.tile_pool(name="h", bufs=2))
            pst = ph.enter_context(tc.tile_pool(name="pst", bufs=4, space="PSUM"))
            for t in range(TT):
                x_t = xpool.tile([128, C], F32, tag="x")
                nc.sync.dma_start(out=x_t, in_=x_r[t])
                negmu, rstd = ln_stats(stats, x_t)
                h_t = hpool.tile([128, C], F32R, tag="h")
                nc.vector.tensor_scalar(h_t, x_t, negmu, rstd,
                                        op0=ALU.add, op1=ALU.mult)
                if ln1_aff:
                    nc.vector.tensor_tensor(h_t, h_t, ln1g_bc, op=ALU.mult)
                    nc.vector.tensor_tensor(h_t, h_t, ln1b_bc, op=ALU.add)
                for g3 in range(2):
                    ps3 = pst.tile([128, 384], F32R, tag="pt")
                    for c3 in range(3):
                        f = g3 * 3 + c3
                        nc.tensor.transpose(ps3[:, c3 * 128:(c3 + 1) * 128],
                                            h_t[:, f * 128:(f + 1) * 128], identity)
                    dst = hT8[:, g3 * 3:(g3 + 1) * 3, t * 128:(t + 1) * 128]
                    if t % 2 == 0:
                        nc.vector.tensor_copy(dst, ps3)
                    else:
                        nc.gpsimd.tensor_copy(dst, ps3)

        wpool = hT_stack.enter_context(tc.tile_pool(name="qkvw", bufs=1))
        wqk8 = [wpool.tile([128, 2, 2 * C], F8, name=f"wqk{j}") for j in range(KC // 2)]
        wv8 = [wpool.tile([128, 2, C], F8, name=f"wv{j}") for j in range(KC // 2)]
        for j in range(KC // 2):
            nc.sync.dma_start(out=wqk8[j], in_=qkvw8_d[j])
            nc.sync.dma_start(out=wv8[j], in_=vw8_d[j])

        # ---------------- Phase B: qkv ----------------
        with nc.named_scope("qkv"), ExitStack() as ph:
            psqk = ph.enter_context(tc.tile_pool(name="psqk", bufs=6, space="PSUM"))
            # q,k feature-major, fp8 at 2^SA; q dense-stacked, k half-zero padded
            for fc in range(2 * KC):
                for tc4 in range(T // 512):
                    ps = psqk.tile([128, 512], F32, tag="ps")
                    for j in range(KC // 2):
                        nc.tensor.matmul(ps, wqk8[j][:, :, fc * 128:(fc + 1) * 128],
                                         hT8[:, 2 * j:2 * j + 2, tc4 * 512:(tc4 + 1) * 512],
                                         start=(j == 0), stop=(j == KC // 2 - 1),
                                         perf_mode=PM.DoubleRow)
                    b = tc4 // 2
                    if fc < KC:      # q chunk: heads (2fc, 2fc+1) dense slot 0
                        hp = fc
                        ls = slice((tc4 % 2) * 512, (tc4 % 2) * 512 + 512)
                        dst = qTd_b[b][hp][:, 0, ls]
                    else:            # k chunk: heads (2hp, 2hp+1), keys on free
                        hp = fc - KC
                        kt0 = (tc4 % 2) * 4
                        dst = kpad_b[b][hp][:, kt0:kt0 + 4, 0, :]
                    eng = nc.vector if (fc + tc4) % 2 == 0 else nc.gpsimd
                    if qkv_bias:
                        eng.tensor_scalar(dst, ps, sqk, qkvb_pp[:, fc:fc + 1],
                                          op0=ALU.mult, op1=ALU.add)
                    else:
                        eng.tensor_scalar_mul(dst, ps, sqk)
            # v token-major -> vp fp8 (kt-paired), natural scale
            for t in range(TT):
                for off, h_lo, h_hi in ((0, 0, 6), (384, 6, 12)):
                    ps = psqk.tile([128, 512], F32, tag="ps")
                    for j in range(KC // 2):
                        nc.tensor.matmul(ps[:, 0:384],
                                         hT8[:, 2 * j:2 * j + 2, t * 128:(t + 1) * 128],
                                         wv8[j][:, :, off:off + 384],
                                         start=(j == 0), stop=(j == KC // 2 - 1),
                                         perf_mode=PM.DoubleRow)
                    dst = vp_b[t // NPB][(t % NPB) // 2][:, t % 2, h_lo:h_hi, 0:HD]
                    eng = nc.vector if t % 2 == 0 else nc.gpsimd
                    if qkv_bias:
                        eng.scalar_tensor_tensor(dst, ps[:, 0:384], iq,
                                                 vb_bc[:, off:off + 384],
                                                 op0=ALU.mult, op1=ALU.add)
                    else:
                        eng.tensor_scalar_mul(dst, ps[:, 0:384], iq)

        for j in range(KC // 2):
            nc.sync.dma_start(out=pw8[j], in_=pw8_d[j])

        # ------- Phases C+E: attention pipelined with the MLP -------
        hT_stack.close()
        pssc_sc = ExitStack()
        pssc = pssc_sc.enter_context(tc.tile_pool(name="pssc", bufs=2, space="PSUM"))
        ln2s = [None] * TT

        def attn_hp(b, hp, psov, pre_pieces, mid_pieces):
            kp = kpad_b[b][hp]
            qd = qTd_b[b][hp]
            oT8 = oT8_b[b]
            for p in pre_pieces:
                p()
            eAs, eBs = [], []
            for kt in range(NPB):
                psS = pssc.tile([128, N], F32, tag="s")
                psB = pssc.tile([128, N], F32, tag="s")
                for qc in range(2):
                    qs = slice(qc * 512, (qc + 1) * 512)
                    nc.tensor.matmul(psS[:, qs], kp[0:64, kt, 0, :],
                                     qd[0:64, 0, qs], start=True, stop=True,
                                     tile_position=(0, 0))
                    nc.tensor.matmul(psB[:, qs], kp[64:128, kt, 0, :],
                                     qd[64:128, 0, qs], start=True, stop=True,
                                     tile_position=(64, 0))
                if kt % 2 == 0:
                    eA_t = epool.tile([128, 2, N], F8, tag="e8", name="eA")
                    eB_t = epool.tile([128, 2, N], F8, tag="e8", name="eB")
                    eAs.append(eA_t)
                    eBs.append(eB_t)
                nc.scalar.activation(eAs[kt // 2][:, kt % 2, :], psS,
                                     AF.Exp, scale=sc_exp, bias=nshift)
                nc.scalar.activation(eBs[kt // 2][:, kt % 2, :], psB,
                                     AF.Exp, scale=sc_exp, bias=nshift)
            for qc in range(2):
                qs = slice(qc * 512, (qc + 1) * 512)
                psoA = psov.tile([128, 512], F32, tag="o")
                psoB = psov.tile([128, 512], F32, tag="o")
                for ps_o, et, h in ((psoA, eAs, 2 * hp), (psoB, eBs, 2 * hp + 1)):
                    for ktp in range(NPB // 2):
                        nc.tensor.matmul(ps_o[0:HD + 4, :],
                                         vp_b[b][ktp][:, :, h, :],
                                         et[ktp][:, :, qs],
                                         start=(ktp == 0), stop=(ktp == 3),
                                         perf_mode=PM.DoubleRow)
                recA = rpool.tile([1, 512], F32R, tag="rA")
                recB = rpool.tile([1, 512], F32R, tag="rB")
                with nc.allow_low_precision(reason="fp32r is fp32 bits"):
                    nc.vector.reciprocal(recA, psoA[HD:HD + 1, :])
                    nc.vector.reciprocal(recB, psoB[HD:HD + 1, :])
                nc.sync.dma_start(out=rec_d[b, hp, qc, 0], in_=recA)
                nc.sync.dma_start(out=rec_d[b, hp, qc, 1], in_=recB)
                bc_sb = bcpool.tile([128, 512], F32R, tag="bc")
                rAd = rec_d[b, hp, qc, 0]
                rBd = rec_d[b, hp, qc, 1]
                nc.sync.dma_start(
                    out=bc_sb[0:64, :],
                    in_=bass.AP(tensor=rAd.tensor, offset=rAd.offset,
                                ap=[[0, 64]] + [list(p) for p in rAd.ap]))
                nc.sync.dma_start(
                    out=bc_sb[64:128, :],
                    in_=bass.AP(tensor=rBd.tensor, offset=rBd.offset,
                                ap=[[0, 64]] + [list(p) for p in rBd.ap]))
                nc.vector.tensor_tensor(
                    oT8[hp // 2][0:64, hp % 2, qs], psoA[0:HD, :],
                    bc_sb[0:64, :], op=ALU.mult)
                nc.gpsimd.tensor_tensor(
                    oT8[hp // 2][64:128, hp % 2, qs], psoB[0:HD, :],
                    bc_sb[64:128, :], op=ALU.mult)
                if qc == 0:
                    for p in mid_pieces:
                        p()

        def proj_b(b, psov, ptag="o"):
            for lt in range(NPB):
                t = b * NPB + lt
                x_t = xres.tile([128, C], F32, tag="xr")
                nc.sync.dma_start(out=x_t, in_=x_r[t])
                for off, sz in ((0, 384), (384, 384)):
                    ps = psov.tile([128, 512], F32, tag=ptag)
                    for j in range(KC // 2):
                        nc.tensor.matmul(ps[:, 0:sz],
                                         oT8_b[b][j][:, :, lt * 128:(lt + 1) * 128],
                                         pw8[j][:, :, off:off + sz],
                                         start=(j == 0), stop=(j == KC // 2 - 1),
                                         perf_mode=PM.DoubleRow)
                    nc.vector.scalar_tensor_tensor(
                        x2t[t][:, off:off + sz], ps[:, 0:sz], ip,
                        x_t[:, off:off + sz], op0=ALU.mult, op1=ALU.add)
                if proj_bias:
                    nc.vector.tensor_tensor(x2t[t], x2t[t], projb_bc, op=ALU.add)

        def stats_b(b):
            for t in range(b * NPB, (b + 1) * NPB):
                st = statsE.tile([128, 12], F32, name=f"stE{t}")
                nc.vector.bn_stats(st[:, 0:6], x2t[t][:, 0:C // 2])
                nc.vector.bn_stats(st[:, 6:12], x2t[t][:, C // 2:C])
                mv = statsE.tile([128, 2], F32, name=f"mvE{t}")
                nc.vector.bn_aggr(mv, st)
                vare = stats2.tile([128, 1], F32, tag="vare")
                nc.vector.tensor_scalar_add(vare, mv[:, 1:2], EPS)
                std = stats2.tile([128, 1], F32, tag="sdE")
                nc.scalar.sqrt(std, vare)
                rstd = statsE.tile([128, 1], F32, name=f"rsE{t}")
                nc.vector.reciprocal(rstd, std)
                negmu = statsE.tile([128, 1], F32, name=f"nmE{t}")
                nc.vector.tensor_scalar_mul(negmu, mv[:, 0:1], -1.0)
                ln2s[t] = (negmu, rstd)

        # ---- attention, both batch items (serial, 4-bank psov) ----
        ph0 = ExitStack()
        psov0 = ph0.enter_context(tc.tile_pool(name="psov0", bufs=4,
                                               space="PSUM"))
        with nc.named_scope("attn0"):
            for hp in range(HP):
                attn_hp(0, hp, psov0, [], [])
            proj_b(0, psov0)
        bstk[0].close()
        stats_b(0)
        with nc.named_scope("attn1"):
            for hp in range(HP):
                attn_hp(1, hp, psov0, [], [])
        bstk[1].close()
        ph0.close()
        pssc_sc.close()

        # ---- MLP over all 4 chunks ----
        CH = 4
        CT = T // CH // 128
        phW = ExitStack()
        w2pool = phW.enter_context(tc.tile_pool(name="w2", bufs=1))
        w2_8, w2r8 = [], []
        for u in range(HC // 2):
            wa = w2pool.tile([128, 2, C], F8, name=f"w2_{u}")
            nc.sync.dma_start(out=wa, in_=w28_d[u])
            w2_8.append(wa)
            wb = w2pool.tile([128, 2, C], F8, name=f"w2r_{u}")
            nc.sync.dma_start(out=wb, in_=w2r8_d[u])
            w2r8.append(wb)
        psf1 = phW.enter_context(tc.tile_pool(name="psf1", bufs=3, space="PSUM"))
        psfT = phW.enter_context(tc.tile_pool(name="psfT", bufs=2, space="PSUM"))
        psf2 = phW.enter_context(tc.tile_pool(name="psf2", bufs=2, space="PSUM"))
        h2pool = phW.enter_context(tc.tile_pool(name="h2", bufs=3))
        h2Tpool = phW.enter_context(tc.tile_pool(name="h2T", bufs=2))
        hrpool = phW.enter_context(tc.tile_pool(name="hr8", bufs=2))
        gpool = phW.enter_context(tc.tile_pool(name="gT", bufs=6))
        g8pool = phW.enter_context(tc.tile_pool(name="g8", bufs=13))
        gr8pool = phW.enter_context(tc.tile_pool(name="gr8", bufs=13))
        outpool = phW.enter_context(tc.tile_pool(name="outp", bufs=2))
        h2T_c, hr_c, g8_c, gr8_c = {}, {}, {}, {}
        def mlp_trans(ch, pspool):
            h2T_c[ch] = h2Tpool.tile([128, KC, CT * 128], F8, tag="h2T",
                                     name=f"h2T{ch}")
            hr_c[ch] = hrpool.tile([128, KC, CT * 128], F8, tag="hr8",
                                   name=f"hr{ch}")
            for lt in range(CT):
                t = ch * CT + lt
                negmu, rstd = ln2s[t]
                h2 = h2pool.tile([128, C], F32R, tag="h2")
                nc.vector.tensor_scalar(h2, x2t[t], negmu, rstd,
                                        op0=ALU.add, op1=ALU.mult)
                if ln2_aff:
                    nc.vector.tensor_tensor(h2, h2, ln2g_bc, op=ALU.mult)
                    nc.vector.tensor_tensor(h2, h2, ln2b_bc, op=ALU.add)
                for g3 in range(2):
                    ps3 = pspool.tile([128, 384], F32R, tag="o")
                    for c3 in range(3):
                        f = g3 * 3 + c3
                        nc.tensor.transpose(ps3[:, c3 * 128:(c3 + 1) * 128],
                                            h2[:, f * 128:(f + 1) * 128], identity)
                    dst8 = h2T_c[ch][:, g3 * 3:(g3 + 1) * 3,
                                     lt * 128:(lt + 1) * 128]
                    if lt % 2 == 0:
                        nc.vector.tensor_copy(dst8, ps3)
                    else:
                        nc.gpsimd.tensor_copy(dst8, ps3)
                    dstr = hr_c[ch][:, g3 * 3:(g3 + 1) * 3,
                                    lt * 128:(lt + 1) * 128]
                    nc.vector.scalar_tensor_tensor(dstr, ps3, 1.0, dst8,
                                                   op0=ALU.mult, op1=ALU.subtract)

        def mlp_fc1(ch, lo, hi):
            if lo == 0:
                g8_c[ch] = [g8pool.tile([128, 2, 512], F8, tag="g8",
                                        name=f"g8_{ch}_{u}")
                            for u in range(HC // 2)]
                gr8_c[ch] = [gr8pool.tile([128, 2, 512], F8, tag="gr8",
                                          name=f"gr8_{ch}_{u}")
                             for u in range(HC // 2)]
            for sidx in range(lo, hi):
                half, hc = sidx // 12, sidx % 12
                ps = psf1.tile([128, 512], F32, tag="f1")
                off = half * (HID // 2) + hc * 128
                nmm = 3 * (KC // 2)
                k = 0
                for j in range(KC // 2):
                    for lh, rh in ((w18[j], h2T_c[ch]), (w18[j], hr_c[ch]),
                                   (w1r8[j], h2T_c[ch])):
                        nc.tensor.matmul(ps, lh[:, :, off:off + 128],
                                         rh[:, 2 * j:2 * j + 2, :],
                                         start=(k == 0), stop=(k == nmm - 1),
                                         perf_mode=PM.DoubleRow)
                        k += 1
                g_t = gpool.tile([128, 512], BF16, tag="g")
                if fc1_bias:
                    nc.scalar.activation(g_t, ps, AF.Gelu, scale=i1,
                                         bias=fc1b_pp[:, sidx:sidx + 1])
                else:
                    nc.scalar.activation(g_t, ps, AF.Gelu, scale=i1)
                g8s = g8_c[ch][sidx // 2][:, sidx % 2, :]
                nc.gpsimd.tensor_copy(g8s, g_t)
                nc.vector.tensor_tensor(gr8_c[ch][sidx // 2][:, sidx % 2, :],
                                        g_t, g8s, op=ALU.subtract)

        def mlp_fc2(ch, pspool):
            g8p, gr8p = g8_c[ch], gr8_c[ch]
            for lt in range(CT):
                t = ch * CT + lt
                out_sb = outpool.tile([128, C], F32, tag="out")
                for off, sz in ((0, 384), (384, 384)):
                    ps = pspool.tile([128, 384], F32, tag="f2")
                    nmm = 3 * (HC // 2)
                    k = 0
                    for u in range(HC // 2):
                        ts = slice(lt * 128, (lt + 1) * 128)
                        for lh, rh in ((g8p[u], w2_8[u]), (gr8p[u], w2_8[u]),
                                       (g8p[u], w2r8[u])):
                            nc.tensor.matmul(ps[:, 0:sz], lh[:, :, ts],
                                             rh[:, :, off:off + sz],
                                             start=(k == 0), stop=(k == nmm - 1),
                                             perf_mode=PM.DoubleRow)
                            k += 1
                    nc.vector.scalar_tensor_tensor(
                        out_sb[:, off:off + sz], ps[:, 0:sz], i2,
                        x2t[t][:, off:off + sz], op0=ALU.mult, op1=ALU.add)
                if fc2_bias:
                    nc.vector.tensor_tensor(out_sb, out_sb, fc2b_bc, op=ALU.add)
                nc.sync.dma_start(out=out_r[t], in_=out_sb)

        with nc.named_scope("mlp"):
            mlp_trans(0, psfT)
            proj_b(1, psf1, ptag="f1")
            stats_b(1)
            mlp_fc1(0, 0, 24)
            mlp_fc2(0, psf2)
            for ch in (1, 2, 3):
                mlp_trans(ch, psfT)
                mlp_fc1(ch, 0, 24)
                mlp_fc2(ch, psf2)
        phW.close()
        bstk1_late.close()
        mid.close()
        attn_sc.close()
        wmlp.close()
        x2s.close()

    nc.finalize()
    return nc


def _get_nc(flags):
    if flags not in _CACHE:
        _CACHE[flags] = _build(flags)
    return _CACHE[flags]


def _scale_for(w):
    return int(np.clip(np.floor(np.log2(200.0 / max(abs(float(w.max())),
                                                    abs(float(w.min())), 1e-9))),
                       0, 14))


def _pack_rows(w, s, residual=False):
    # [K, M] f32 -> [K//256, 128, 2, M] fp8 at scale 2**s (+ residual fp8)
    K, M = w.shape
    ws = (w * float(2 ** s)).reshape(K // 256, 2, 128, M).transpose(0, 2, 1, 3)
    w8 = np.ascontiguousarray(ws).astype(NP8)
    if not residual:
        return w8
    wr8 = np.ascontiguousarray(ws - w8.astype(np.float32)).astype(NP8)
    return w8, wr8


def kernel(**inputs):
    inp = {k: np.ascontiguousarray(np.asarray(v, dtype=np.float32))
           for k, v in inputs.items()}
    s_qkv = _scale_for(inp["qkv_w"])
    s_proj = _scale_for(inp["proj_w"])
    s_fc1 = _scale_for(inp["fc1_w"])
    s_fc2 = _scale_for(inp["fc2_w"])
    flags = (
        not (np.all(inp["ln1_g"] == 1.0) and np.all(inp["ln1_b"] == 0.0)),
        not (np.all(inp["ln2_g"] == 1.0) and np.all(inp["ln2_b"] == 0.0)),
        bool(np.any(inp["qkv_b"] != 0.0)),
        bool(np.any(inp["proj_b"] != 0.0)),
        bool(np.any(inp["fc1_b"] != 0.0)),
        bool(np.any(inp["fc2_b"] != 0.0)),
        s_qkv, s_proj, s_fc1, s_fc2,
    )
    nc = _get_nc(flags)
    x = inp["x"]
    shared = {k: v for k, v in inp.items()
              if k not in ("x", "qkv_w", "proj_w", "fc1_w", "fc2_w")}
    shared["qkv_w8"] = _pack_rows(inp["qkv_w"][:, 0:2 * C], s_qkv)
    shared["v_w8"] = _pack_rows(inp["qkv_w"][:, 2 * C:3 * C], s_qkv)
    shared["proj_w8"] = _pack_rows(inp["proj_w"], s_proj)
    shared["fc1_w8"], shared["fc1_wr8"] = _pack_rows(inp["fc1_w"], s_fc1,
                                                     residual=True)
    shared["fc2_w8"], shared["fc2_wr8"] = _pack_rows(inp["fc2_w"], s_fc2,
                                                     residual=True)
    in_maps = [dict(shared, x=x[i * BL:(i + 1) * BL]) for i in range(NCORES)]
    res = run_bass_kernel_spmd(nc, in_maps, core_ids=list(range(NCORES)))
    out = np.concatenate([res.results[i]["out"] for i in range(NCORES)], axis=0)
    return out.astype(np.float32)
